# revision 1
# baseline (speedup 1.0000x reference)
"""PWC-Net local correlation (MD=4, 81 displacements) on 8 Trainium2 cores.

Problem: t1, t2: [B=4, C=128, H=128, W=256] fp32
  out[b, d, y, x] = mean_c t1[b,c,y,x] * t2pad[b,c,y+dy,x+dx],  d = (dy+4)*9+(dx+4)

Sharding: 8 cores = B(4) x W-half(2); inputs pre-sliced/padded/bf16-cast on
host (t1 pre-scaled by 1/C so the gram is already the mean).

Per core, per output column x0 (128 of them):
  1. Column-gram on TensorE: stationary lhsT = t1[:, :, x0] (C x H), moving
     rhs = t2pad[:, y'-window, x0-4..x0+4] (C x 138*9, 3 chunks of 414 cols).
     Gram G[y, y'*9+dx'] holds all 81 displacement dot-products for every
     output row y at column x0, on the skewed band G[y, 9y+d], d=0..80.
  2. ACT/DVE copy PSUM->SBUF bf16 (gsb, batched 8 columns).
  3. Band extraction via a DRAM bounce (DGE descriptors cover only 4
     partitions, so skewed SBUF reads are inconsistent; and HWDGE costs
     ~0.6us per DMA instruction, so few, large DMAs): write the 360-wide
     group-rebased band per 32-row group to DRAM with regular APs (4 DMAs
     per 8-column batch), then ONE readback per batch with the whole skew
     expressed on the DRAM side: pt8[y, x0*81+d] = G[y, 9y+d].
  4. One pixel-major DMA per batch: outp[(y*128+x0)*81+d]. Host unshards:
     transpose (y, x, d) -> (d, y, x) and cast fp32.

Steady state is PSUM-evacuation-bound: 1242 copy-elements per column through
ACT (1.2 GHz) + DVE (0.96 GHz) at 1 elem/partition/cycle; PE/HWDGE/DMA sit
below that. Explored-and-rejected next steps, for the record:
  - DRAM->DRAM skewed DMA (skips the pt8 staging, modeled -3.5us): compiles
    but is fatal on hardware (NRT_EXEC_UNIT_UNRECOVERABLE). Do not revisit.
  - 2D-patch grams (lhsT = 4-col x 32-row pixel blocks, 480-col windows):
    cuts copy work 2.6x (3.75 vs 9.7 elems/pixel) and PE 2.4x. The skew
    becomes +1/partition intra-quad (expressible via the DGE 4-partition
    descriptor wrap at offset 0) + 12/row inter-group (bakeable into a DRAM
    band with row stride S=480K+12, group stride 4S-12, readback
    [[S,128],[1,108]] + host 81-of-108 gather). But the band dump must carry
    the full 480-col gram rows, so DMA bytes inherit exactly what the
    engines save (~15.7 MB/core) - break-even under the cost model.
"""

import numpy as np
import ml_dtypes

B, C, H, W = 4, 128, 128, 256
MD = 4
D = (2 * MD + 1) ** 2  # 81
WH = W // 2  # 128 columns per core
YP = 138  # padded y' rows: 4 + 128 + 4 + 2 junk (uniform 3x46 chunking)
XP = 136  # padded x window: 128 + 2*4
NCHUNK = 46 * 9  # 414 columns per matmul (one psum bank)
GSTRIDE = 3 * NCHUNK  # 1242 gram columns per x0
XBATCH = 8  # x0 columns per gsb staging tile
GROWS = 32  # band group rows
BW32 = 9 * (GROWS - 1) + D  # 360: band width per 32-row group
RSTRIDE = XBATCH * BW32  # 2880: band row stride
GS32 = GROWS * (RSTRIDE + 9)  # 92448: band group stride (enables merged readback)
_compiled = None


def _build(reps=None):
    """Build the per-core program. reps=None: single pass. reps=R: wrap the
    compute in a hardware For loop (identical output each rep) — used only
    for benchmarking so wall-clock deltas resolve the kernel time through
    the noisy RPC dispatch floor."""
    import concourse.bacc as bacc
    import concourse.bass as bass
    import concourse.mybir as mybir
    import concourse.tile as tile

    bf = mybir.dt.bfloat16
    nc = bacc.Bacc("TRN2", target_bir_lowering=False, debug=False, num_devices=8)
    t1s = nc.dram_tensor("t1s", [C, H * WH], bf, kind="ExternalInput").ap()
    t2s = nc.dram_tensor("t2s", [C, 4 * YP * 40], bf, kind="ExternalInput").ap()
    outp = nc.dram_tensor("outp", [H * WH * D], bf, kind="ExternalOutput").ap()

    with tile.TileContext(nc) as tc:
        with (
            tc.tile_pool(name="inputs", bufs=1) as inp,
            tc.tile_pool(name="gpool", bufs=3) as gpool,
            tc.tile_pool(name="ptp", bufs=4) as ptp,
            tc.tile_pool(name="psumA", bufs=2, space="PSUM") as ppa,
            tc.tile_pool(name="psumB", bufs=4, space="PSUM") as ppb,
            tc.tile_pool(name="dram", bufs=3, space="DRAM") as dp,
        ):
            # inputs staged as 4 x-block tiles each (32 output columns +
            # 8-col t2 halo) so the first batch's matmuls start after ~8us
            # of loads instead of waiting for the full 9 MB
            t1t, t2t = [], []
            for i in range(4):
                a = inp.tile([C, H * 32], bf, name=f"t1t_{i}")
                nc.sync.dma_start(
                    a[:], bass.AP(t1s.tensor, H * 32 * i, [[H * WH, C], [1, H * 32]])
                )
                t1t.append(a)
                bt = inp.tile([C, YP * 40], bf, name=f"t2t_{i}")
                nc.sync.dma_start(
                    bt[:], bass.AP(t2s.tensor, YP * 40 * i, [[4 * YP * 40, C], [1, YP * 40]])
                )
                t2t.append(bt)
            S1t = t1t[0].tensor.shape[-1]
            S2t = t2t[0].tensor.shape[-1]

            def batch_loop(_iv=None):
                for b8 in range(WH // XBATCH):  # 16 batches of 8 columns
                        # even columns staged by ACT into gsbA, odd by DVE into gsbB:
                        # separate tiles so Tile never sees a cross-engine WAW, and
                        # each engine does one full-gram copy per column
                        gsbA = gpool.tile([C, (XBATCH // 2) * GSTRIDE], bf, name="gsbA")
                        gsbB = gpool.tile([C, (XBATCH // 2) * GSTRIDE], bf, name="gsbB")
                        for j in range(XBATCH):
                            x0 = b8 * XBATCH + j
                            psa = ppa.tile([128, 1024], mybir.dt.float32, name="psa")
                            psb = ppb.tile([128, 512], mybir.dt.float32, name="psb")
                            SpA = psa.tensor.shape[-1]
                            ib, xl = x0 // 32, x0 % 32
                            lhsT = bass.AP(t1t[ib].tensor, xl, [[S1t, C], [32, H]])
                            for k in range(3):
                                rhs = bass.AP(
                                    t2t[ib].tensor, 46 * k * 40 + xl, [[S2t, C], [40, 46], [1, 9]]
                                )
                                if k < 2:
                                    out_ap = bass.AP(psa.tensor, 512 * k, [[SpA, 128], [1, NCHUNK]])
                                else:
                                    out_ap = psb[:, 0:NCHUNK]
                                nc.tensor.matmul(out_ap, lhsT, rhs, start=True, stop=True)
                            # two copies per column (chunks 0+1 can start before MM2),
                            # one engine per column; roles swap each batch for balance
                            slot = j // 2
                            gt = gsbA if j % 2 == 0 else gsbB
                            eng = nc.scalar.copy if (j + b8) % 2 == 0 else nc.vector.tensor_copy
                            eng(
                                gt[:, slot * GSTRIDE : slot * GSTRIDE + 2 * NCHUNK],
                                bass.AP(psa.tensor, 0, [[SpA, 128], [512, 2], [1, NCHUNK]]),
                            )
                            eng(
                                gt[:, slot * GSTRIDE + 2 * NCHUNK : (slot + 1) * GSTRIDE],
                                psb[:, 0:NCHUNK],
                            )
                        # 360-wide rebased band per 32-row group -> DRAM (regular APs),
                        # one DMA per (group, parity); dst x0-stride 720 re-interleaves
                        band = dp.tile([(GROWS - 1) * (RSTRIDE + 9) + RSTRIDE + 3 * GS32], bf, name="band")
                        SgA = gsbA.tensor.shape[-1]
                        SgB = gsbB.tensor.shape[-1]
                        for g in range(H // GROWS):
                            for par, (gt, Sgx) in enumerate(((gsbA, SgA), (gsbB, SgB))):
                                nc.sync.dma_start(
                                    bass.AP(band.tensor, g * GS32 + par * BW32,
                                            [[RSTRIDE, GROWS], [2 * BW32, XBATCH // 2], [1, BW32]]),
                                    bass.AP(gt.tensor, GROWS * g * Sgx + 9 * GROWS * g,
                                            [[Sgx, GROWS], [GSTRIDE, XBATCH // 2], [1, BW32]]),
                                )
                        # one skewed readback for the whole batch: pt8[y, j*81+d] = G[y, 9y+d]
                        # (DRAM->DRAM DMA would skip this staging but is fatal on HW:
                        # NRT_EXEC_UNIT_UNRECOVERABLE; keep the SBUF bounce)
                        pt8 = ptp.tile([128, XBATCH * D], bf, name="pt8")
                        nc.gpsimd.dma_start(
                            pt8[:],
                            bass.AP(band.tensor, 0, [[RSTRIDE + 9, H], [BW32, XBATCH], [1, D]]),
                        )
                        nc.gpsimd.dma_start(
                            bass.AP(outp.tensor, b8 * XBATCH * D, [[WH * D, H], [1, XBATCH * D]]),
                            pt8[:],
                        )

            if reps is None:
                batch_loop()
            else:
                with tc.For_i(0, reps, 1) as iv:
                    batch_loop(iv)

    nc.compile()
    return nc


def _prep_inputs(t1, t2):
    bf16 = ml_dtypes.bfloat16
    in_maps = []
    for k in range(8):
        b, xh = k // 2, k % 2
        xs = xh * WH
        t1c = (t1[b, :, :, xs : xs + WH] * (1.0 / C)).astype(bf16)
        t1blk = np.concatenate(
            [t1c[:, :, 32 * i : 32 * i + 32].reshape(C, H * 32) for i in range(4)], axis=1
        )
        t2p = np.zeros((C, YP, XP), dtype=bf16)
        lo, hi = max(0, xs - MD), min(W, xs + WH + MD)
        t2p[:, MD : MD + H, lo - (xs - MD) : hi - (xs - MD)] = t2[b, :, :, lo:hi].astype(bf16)
        t2blk = np.concatenate(
            [t2p[:, :, 32 * i : 32 * i + 40].reshape(C, YP * 40) for i in range(4)], axis=1
        )
        in_maps.append({"t1s": t1blk, "t2s": t2blk})
    return in_maps


def kernel(t1: np.ndarray, t2: np.ndarray) -> np.ndarray:
    from concourse.bass_utils import run_bass_kernel_spmd

    global _compiled
    if _compiled is None:
        _compiled = _build()
    nc = _compiled

    t1 = np.asarray(t1, dtype=np.float32)
    t2 = np.asarray(t2, dtype=np.float32)
    res = run_bass_kernel_spmd(nc, _prep_inputs(t1, t2), list(range(8)))

    out = np.empty((B, D, H, W), dtype=np.float32)
    for k in range(8):
        b, xh = k // 2, k % 2
        xs = xh * WH
        pix = res.results[k]["outp"].astype(np.float32).reshape(H, WH, D)
        out[b, :, :, xs : xs + WH] = pix.transpose(2, 0, 1)
    return out



# revision 2
# speedup vs baseline: 2.4918x; 2.4918x over previous
"""PWC-Net local correlation (MD=4, 81 displacements) on 8 Trainium2 cores, v8.

Problem: t1, t2: [B=4, C=128, H=128, W=256] fp32
  out[b, d, y, x] = mean_c t1[b,c,y,x] * t2pad[b,c,y+dy,x+dx],  d = (dy+4)*9+(dx+4)

Sharding: 8 cores = B(4) x W-half(2); host pre-slices/pads/bf16-casts
(t1 pre-scaled by 1/C so the gram is already the mean).

Per core (128x128 pixels), patch-gram architecture:
  - image tiled into 128 blocks of 16x8 pixels; per block ONE matmul:
    stationary lhsT = t1 block pixels (C x 128, partition p = 8*r + c),
    moving rhs = t2 window (24x16 = 384 cols, via a 3-dim AP into the
    full padded t2 tile). Gram [128 pix, 384] in one PSUM bank.
  - ACT/DVE copies evacuate PSUM fp32 -> shared gsb bf16 tile,
    32 blocks column-interleaved (dst stride 32) so dump descriptors are
    large/contiguous. Subtile deps let both engines share one tile.
  - Band dump: per 16-partition group (2 pixel rows), the needed gram
    columns are the contiguous 160-wide (10 window rows x 16) band
    starting at wcol 32*g'. One HWDGE DMA per (batch, group) with the
    group rebase in the scalar offset (per-partition skew is illegal in
    SBUF-side AP dims; scalar offsets may mix partition+column).
  - The band IS the kernel output (1.98x inflated vs the final 81/pixel);
    host unshard finishes with a single as_strided gather per core
    (pure indexing - every output value is computed exactly once on
    device; host only selects/arranges, like the baseline's transpose).
This removes the baseline's DRAM bounce (band readback + pixel-major
rewrite) entirely: ~14MB DMA/core instead of ~27MB, 40 HWDGE DMAs
instead of 136, and 3.2x less PE + PSUM-evacuation work.
"""

import numpy as np
import ml_dtypes

B, C, H, W = 4, 128, 128, 256
MD = 4
D = (2 * MD + 1) ** 2  # 81
WH = W // 2  # 128 columns per core
BR, BC = 16, 8  # block pixel rows/cols
NBY, NBX = H // BR, WH // BC  # 8 x 16 = 128 blocks
IL = 32  # blocks per batch (interleave factor)
NBATCH = (NBY * NBX) // IL  # 4
WR, WC = BR + 2 * MD, BC + 2 * MD  # 24 x 16 window
GW = WR * WC  # 384 gram width
T2R = H + 2 * MD  # 136 padded t2 rows
T2C = WH + 2 * MD  # 136 padded t2 cols
SG = GW * IL  # 12288 gsb row width
RPG = 2  # pixel rows per 16-partition dump group
NG = 128 // (RPG * BC)  # 8 groups
BW = (RPG + 2 * MD) * WC  # 160 band width per partition
BDW = BW * IL  # 5120 interleaved band width
GBYTES = 16 * BDW  # 81920 elems per (batch, group) dump
OUTN = NBATCH * NG * GBYTES  # 2621440 elems total
_compiled = None


def _build(reps=None):
    """Build the per-core program. reps=None: single pass. reps=R wraps the
    compute in a hardware For loop (benchmarking only)."""
    import concourse.bacc as bacc
    import concourse.bass as bass
    import concourse.mybir as mybir
    import concourse.tile as tile

    bf = mybir.dt.bfloat16
    nc = bacc.Bacc("TRN2", target_bir_lowering=False, debug=False, num_devices=8)
    t1s = nc.dram_tensor("t1s", [C, H * WH], bf, kind="ExternalInput").ap()
    t2s = nc.dram_tensor("t2s", [C, T2R * T2C], bf, kind="ExternalInput").ap()
    outp = nc.dram_tensor("outp", [OUTN], bf, kind="ExternalOutput").ap()

    with tile.TileContext(nc) as tc:
        with (
            tc.tile_pool(name="inputs", bufs=1) as inp,
            tc.tile_pool(name="gpool", bufs=2) as gpool,
            tc.tile_pool(name="psum", bufs=8, space="PSUM") as pp,
        ):
            # one t1 tile + one t2 tile, each loaded in 4 batch-aligned
            # chunks so batch b's matmuls only wait on the chunks they read
            # (subtile deps): t1 chunk b = blocks 32b..32b+31; t2 chunk
            # rows [0:40], [40:72], [72:104], [104:136].
            t1t = inp.tile([C, H * WH], bf, name="t1t")
            t2t = inp.tile([C, T2R * T2C], bf, name="t2t")
            t2rows = [(0, 40), (40, 72), (72, 104), (104, 136)]
            for b in range(NBATCH):
                r0, r1 = t2rows[b]
                nc.sync.dma_start(
                    bass.AP(t2t.tensor, r0 * T2C, [[T2R * T2C, C], [1, (r1 - r0) * T2C]]),
                    bass.AP(t2s.tensor, r0 * T2C, [[T2R * T2C, C], [1, (r1 - r0) * T2C]]),
                )
                nc.sync.dma_start(
                    bass.AP(t1t.tensor, IL * 128 * b, [[H * WH, C], [1, IL * 128]]),
                    bass.AP(t1s.tensor, IL * 128 * b, [[H * WH, C], [1, IL * 128]]),
                )

            def batch_loop(_iv=None):
                for b in range(NBATCH):
                    gsb = gpool.tile([C, SG], bf, name="gsb")
                    for j in range(IL):
                        blk = IL * b + j
                        rb, cb = blk // NBX, blk % NBX
                        ps = pp.tile([128, 512], mybir.dt.float32, name="ps")
                        lhsT = bass.AP(t1t.tensor, blk * 128, [[H * WH, C], [1, 128]])
                        rhs = bass.AP(
                            t2t.tensor,
                            (BR * rb) * T2C + BC * cb,
                            [[T2R * T2C, C], [T2C, WR], [1, WC]],
                        )
                        nc.tensor.matmul(ps[:, 0:GW], lhsT, rhs, start=True, stop=True)
                        # ACT is faster per element; give it 17 of 32 blocks
                        eng = nc.scalar.copy if (j % 2 == 0 or j == 1) else nc.vector.tensor_copy
                        eng(
                            bass.AP(gsb.tensor, j, [[SG, 128], [IL, GW]]),
                            ps[:, 0:GW],
                        )
                    # one dump per 16-partition group: contiguous 5120-elem
                    # runs; group rebase (partitions 16g', band start wcol
                    # 32g' -> col 1024g') lives in the scalar offset.
                    for g in range(NG):
                        nc.sync.dma_start(
                            bass.AP(outp.tensor, (b * NG + g) * GBYTES,
                                    [[BDW, 16], [1, BDW]]),
                            bass.AP(gsb.tensor, 16 * g * SG + (RPG * WC * IL) * g,
                                    [[SG, 16], [1, BDW]]),
                        )

            if reps is None:
                batch_loop()
            else:
                with tc.For_i(0, reps, 1) as iv:
                    batch_loop(iv)

    nc.compile()
    return nc


def _prep_inputs(t1, t2):
    bf16 = ml_dtypes.bfloat16
    in_maps = []
    for k in range(8):
        b, xh = k // 2, k % 2
        xs = xh * WH
        t1c = (t1[b, :, :, xs : xs + WH] * (1.0 / C)).astype(bf16)
        # block-major: t1blk[c, ((rb*NBX + cb)*BR + r)*BC + cx]
        t1blk = np.ascontiguousarray(
            t1c.reshape(C, NBY, BR, NBX, BC).transpose(0, 1, 3, 2, 4)
        ).reshape(C, H * WH)
        t2p = np.zeros((C, T2R, T2C), dtype=bf16)
        lo, hi = max(0, xs - MD), min(W, xs + WH + MD)
        t2p[:, MD : MD + H, lo - (xs - MD) : hi - (xs - MD)] = t2[b, :, :, lo:hi].astype(
            bf16
        )
        in_maps.append({"t1s": t1blk, "t2s": t2p.reshape(C, T2R * T2C)})
    return in_maps


def _unshard(band):
    """band: bf16 [OUTN] for one core -> fp32 [81, H, WH]."""
    it = band.dtype.itemsize
    # axes: (b, g', r'', c, dy, dx, jhi, jlo)
    v = np.lib.stride_tricks.as_strided(
        band,
        shape=(NBATCH, NG, RPG, BC, 2 * MD + 1, 2 * MD + 1, NBY // NBATCH, NBX),
        strides=tuple(
            s * it
            for s in (
                NG * GBYTES,          # batch
                GBYTES,               # group
                BC * BDW + WC * IL,   # r'': 8 partitions + 16 wcols
                BDW + IL,             # c: 1 partition + 1 wcol
                WC * IL,              # dy: 16 wcols
                IL,                   # dx: 1 wcol
                16,                   # jhi: j += 16
                1,                    # jlo: j += 1
            )
        ),
    )
    a = v.astype(np.float32)
    # y = 32b + 16jhi + 2g' + r''; x = 8jlo + c; d = 9dy + dx
    return (
        a.transpose(4, 5, 0, 6, 1, 2, 7, 3)
        .reshape(D, H, WH)
    )


def kernel(t1: np.ndarray, t2: np.ndarray) -> np.ndarray:
    from concourse.bass_utils import run_bass_kernel_spmd

    global _compiled
    if _compiled is None:
        _compiled = _build()
    nc = _compiled

    t1 = np.asarray(t1, dtype=np.float32)
    t2 = np.asarray(t2, dtype=np.float32)
    res = run_bass_kernel_spmd(nc, _prep_inputs(t1, t2), list(range(8)))

    out = np.empty((B, D, H, W), dtype=np.float32)
    for k in range(8):
        b, xh = k // 2, k % 2
        xs = xh * WH
        out[b, :, :, xs : xs + WH] = _unshard(res.results[k]["outp"])
    return out
